# revision 1
# baseline (speedup 1.0000x reference)
"""Trainium2 Bass kernel for nn_MemoryOnGpu (retrieval_knn).

Reference semantics: for each (query q, dataset d, bucket n):
  pick b* = argmax_b  <query[q,d,:], key_db[d, b*128+n, :]>   (b in [0, DB/128))
  output selected_keys[q,d,n,:]   = key_db[d, b**128+n, :]
  output selected_values[q,d,n,:] = value_db[d, b**128+n, :]
(The reference's softmax(scores*1e6) is a hard argmax except for
near-exact ties, and its all-zero-row mask never fires for this input.)

Sharding: one dataset per NeuronCore (8 datasets, 8 cores) — fully
independent retrievals.

Per-core pipeline (per 128-query chunk, streaming the key matrix once
per chunk in a bucket-inner column order so every PSUM tile contains the
complete bucket-size reduction range for its buckets):
  PE   : fp32 matmul  scores[128q, (n, b)] = qT^T @ kTp
  ACT  : stage PSUM -> SBUF (exact fp32 copy)
  DVE  : reduce-max over b (per n)          -> M
  GPSIMD: eq = (scores >= M)                -> one-hot (bf16)
  DVE  : eq * (b*128) iota, reduce-max      -> b**128
  DVE  : + n iota, convert int32            -> kv row offsets
  SWDGE: indirect DMA gather of interleaved [key|value] 512B rows
  HWDGE: linear write of gathered rows to HBM
Host re-splits the interleaved kv output into keys/values.
"""

import sys

import numpy as np

for _p in ("/opt/trn_rl_repo", "/root/.axon_site/_ro/trn_rl_repo"):
    if _p not in sys.path:
        sys.path.insert(0, _p)

NUM_QUERIES = 1024
NUM_DATASETS = 8
DB_SIZE = 32768
KEY_FEATURES = 64
VALUE_FEATURES = 64
NUM_NEIGHBORS = 128  # == num_buckets == partition count of the n axis

_NC_CACHE = {}


def build_nc(Q=NUM_QUERIES, DB=DB_SIZE):
    """Build the single-core Bass program (same program on all cores)."""
    import concourse.bass as bass
    import concourse.mybir as mybir
    import concourse.tile as tile
    from concourse import bacc

    F = KEY_FEATURES
    NB = NUM_NEIGHBORS          # buckets (= output neighbors)
    BS = DB // NB               # bucket size (argmax range)
    KVW = 128                   # interleaved kv row width (64 key + 64 value)
    NN = 512 // BS              # n's per 512-wide psum tile
    assert 512 % BS == 0 and NB % NN == 0 and Q % 128 == 0
    T = NB // NN                # tiles per q-chunk
    QC = Q // 128               # q-chunks
    NCOLS = NB * BS             # = DB
    KCH = min(4096, NCOLS)      # kTp streaming chunk (columns)
    NBH = NB // 2               # gather half

    nc = bacc.Bacc()
    qT = nc.declare_dram_parameter("qT", [F, Q], mybir.dt.float32, isOutput=False)
    kTp = nc.declare_dram_parameter("kTp", [F, NCOLS], mybir.dt.float32, isOutput=False)
    kv = nc.declare_dram_parameter("kv", [DB, KVW], mybir.dt.float32, isOutput=False)
    biota = nc.declare_dram_parameter("biota", [128, 512], mybir.dt.bfloat16, isOutput=False)
    niota = nc.declare_dram_parameter("niota", [128, NB], mybir.dt.float32, isOutput=False)
    identity = nc.declare_dram_parameter("identity", [128, 128], mybir.dt.float32, isOutput=False)
    okv = nc.declare_dram_parameter("okv", [Q, NB, KVW], mybir.dt.float32, isOutput=True)

    X = mybir.AxisListType.X
    OP = mybir.AluOpType

    with tile.TileContext(nc) as tc:
        with (
            tc.tile_pool(name="const", bufs=1) as constp,
            tc.tile_pool(name="kst", bufs=3) as kstp,
            tc.tile_pool(name="stage", bufs=4) as stp,
            tc.tile_pool(name="small", bufs=6) as smp,
            tc.tile_pool(name="acc", bufs=2) as accp,
            tc.tile_pool(name="gkv", bufs=2) as gkvp,
            tc.tile_pool(name="ps", bufs=4, space="PSUM") as psp,
        ):
            qt = constp.tile([F, Q], mybir.dt.float32, tag="qt")
            nc.sync.dma_start(out=qt[:], in_=qT[:])
            bio = constp.tile([128, 512], mybir.dt.bfloat16, tag="bio")
            nc.sync.dma_start(out=bio[:], in_=biota[:])
            nio = constp.tile([128, NB], mybir.dt.float32, tag="nio")
            nc.sync.dma_start(out=nio[:], in_=niota[:])
            ident = constp.tile([128, 128], mybir.dt.float32, tag="ident")
            nc.sync.dma_start(out=ident[:], in_=identity[:])
            for qc in range(QC):
                bsel = accp.tile([128, NB], mybir.dt.float32, tag="bsel")
                for kc in range(NCOLS // KCH):
                    kt = kstp.tile([F, KCH], mybir.dt.float32, tag="kt")
                    nc.sync.dma_start(out=kt[:], in_=kTp[:, kc * KCH:(kc + 1) * KCH])
                    for tt in range(KCH // 512):
                        t = kc * (KCH // 512) + tt
                        ps = psp.tile([128, 512], mybir.dt.float32, tag="ps")
                        nc.tensor.matmul(
                            ps[:],
                            qt[:, qc * 128:(qc + 1) * 128],
                            kt[:, tt * 512:(tt + 1) * 512],
                            start=True,
                            stop=True,
                        )
                        stv = ps[:].rearrange("p (n b) -> p n b", b=BS)
                        # per-(q,n) max over b (DVE, straight from PSUM)
                        msl = smp.tile([128, NN], mybir.dt.float32, tag="msl")
                        nc.vector.tensor_reduce(msl[:], stv, axis=X, op=OP.max)
                        # one-hot at the max, written into a 4-tile batch
                        # buffer so the mult/reduce below amortize their
                        # per-instruction dispatch overhead 4x.
                        mb = bass.AP(msl[:].tensor, msl[:].offset, [*msl[:].ap, [0, BS]])
                        if t % 4 == 0:
                            eq4 = stp.tile([128, 2048], mybir.dt.bfloat16, tag="eq4")
                        sl = (t % 4) * 512
                        nc.vector.tensor_tensor(
                            out=eq4[:, sl:sl + 512].rearrange("p (n b) -> p n b", b=BS),
                            in0=stv,
                            in1=mb,
                            op=OP.is_ge,
                        )
                        if t % 4 == 3:
                            # index extraction: max(eq * b*128) over b
                            b4 = bass.AP(bio[:].tensor, bio[:].offset,
                                         [bio[:].ap[0], [0, 4], [1, 512]])
                            ebi = stp.tile([128, 2048], mybir.dt.bfloat16, tag="ebi")
                            nc.vector.tensor_tensor(
                                out=ebi[:].rearrange("p (a j) -> p a j", j=512),
                                in0=eq4[:].rearrange("p (a j) -> p a j", j=512),
                                in1=b4,
                                op=OP.mult,
                            )
                            nc.vector.tensor_reduce(
                                bsel[:, (t - 3) * NN:(t + 1) * NN],
                                ebi[:].rearrange("p (x b) -> p x b", b=BS),
                                axis=X,
                                op=OP.max,
                            )
                # kv row index = b*128 + n
                offf = accp.tile([128, NB], mybir.dt.float32, tag="offf")
                nc.vector.tensor_tensor(out=offf[:], in0=bsel[:], in1=nio[:], op=OP.add)
                # Gather: the only indirect-DMA pattern this NRT handles
                # correctly is the canonical one-offset-per-partition form
                # (BEDROCK image: no GPSIMD ucode libs, so dma_gather is
                # unavailable). One gather per bucket n: 128 rows, one per
                # query partition, then a strided 64KB write to HBM.
                offi = accp.tile([128, NB], mybir.dt.int32, tag="offi")
                nc.vector.tensor_copy(out=offi[:], in_=offf[:])
                for n in range(NB):
                    gk = gkvp.tile([128, KVW], mybir.dt.float32, tag="gk")
                    nc.gpsimd.indirect_dma_start(
                        out=gk[:],
                        out_offset=None,
                        in_=kv[:],
                        in_offset=bass.IndirectOffsetOnAxis(
                            ap=offi[:, n:n + 1], axis=0
                        ),
                    )
                    nc.sync.dma_start(
                        out=okv[qc * 128:(qc + 1) * 128, n, :],
                        in_=gk[:],
                    )
    if not nc.is_finalized():
        nc.finalize()  # Bacc: reg alloc + split multi-sem waits for TRN2
    return nc


def _get_nc(Q, DB):
    key = (Q, DB)
    if key not in _NC_CACHE:
        _NC_CACHE[key] = build_nc(Q, DB)
    return _NC_CACHE[key]


def make_core_inputs(query, key_db, value_db, d, Q, DB):
    """Host-side prep of one core's input arrays (dataset d)."""
    import ml_dtypes

    F = KEY_FEATURES
    NB = NUM_NEIGHBORS
    BS = DB // NB
    qTn = np.ascontiguousarray(query[:, d, :].T.astype(np.float32))          # (F, Q)
    # kTp[f, n*BS + b] = key_db[d, b*NB + n, f]  (bucket-inner column order)
    kperm = key_db[d].reshape(BS, NB, F).transpose(2, 1, 0)                   # (F, NB, BS)
    kTpn = np.ascontiguousarray(kperm.reshape(F, NB * BS).astype(np.float32))
    kvn = np.ascontiguousarray(
        np.concatenate([key_db[d], value_db[d]], axis=1).astype(np.float32)  # (DB, 128)
    )
    j = np.arange(512)
    brow = ((j % BS) * NB).astype(ml_dtypes.bfloat16)
    bio = np.broadcast_to(brow, (128, 512)).copy()
    nio = np.broadcast_to(np.arange(NB, dtype=np.float32), (128, NB)).copy()
    ident = np.eye(128, dtype=np.float32)
    return {"qT": qTn, "kTp": kTpn, "kv": kvn, "biota": bio, "niota": nio,
            "identity": ident}


def kernel(query, key_db, value_db, num_neighbors):
    from concourse.bass_utils import run_bass_kernel_spmd

    query = np.asarray(query, dtype=np.float32)
    key_db = np.asarray(key_db, dtype=np.float32)
    value_db = np.asarray(value_db, dtype=np.float32)
    assert int(num_neighbors) == NUM_NEIGHBORS
    Q, D, F = query.shape
    _, DB, _ = key_db.shape
    assert (Q, D, F, DB) == (NUM_QUERIES, NUM_DATASETS, KEY_FEATURES, DB_SIZE)

    nc = _get_nc(Q, DB)
    in_maps = [make_core_inputs(query, key_db, value_db, d, Q, DB) for d in range(D)]
    res = run_bass_kernel_spmd(nc, in_maps, core_ids=list(range(D)))

    sel_k = np.empty((Q, D, NUM_NEIGHBORS, KEY_FEATURES), dtype=np.float32)
    sel_v = np.empty((Q, D, NUM_NEIGHBORS, VALUE_FEATURES), dtype=np.float32)
    for d in range(D):
        okv = res.results[d]["okv"]                      # (Q, NB, 128)
        sel_k[:, d] = okv[:, :, :KEY_FEATURES]
        sel_v[:, d] = okv[:, :, KEY_FEATURES:]
    return sel_k, sel_v



# revision 5
# speedup vs baseline: 2.0809x; 2.0809x over previous
"""Trainium2 Bass kernel for nn_MemoryOnGpu (retrieval_knn).

Reference semantics: for each (query q, dataset d, bucket n):
  pick b* = argmax_b  <query[q,d,:], key_db[d, b*128+n, :]>   (b in [0, 256))
  selected_keys[q,d,n,:]   = key_db[d, b**128+n, :]
  selected_values[q,d,n,:] = value_db[d, b**128+n, :]

Sharding: one dataset per NeuronCore (8 datasets, 8 cores).

Per-core pipeline, per 128-query chunk, per 2048-column super-tile
(8 buckets, columns in bucket-inner order so each bucket's 256
candidates are contiguous):
  PE  : fp32 matmul  scores[128q, 2048] = qT^T @ kT    (exact)
  DVE : segmented reduce-max, negated    -> Mneg[q, n]
  ACT : per-bucket Sign(scores + Mneg)   -> eq' in {-1, 0}   (0 at argmax)
  DVE : per-bucket fused STT (eq'+1)*biota with accum-sum -> bsel = b*/256
  DVE : clamp + decode to kv row index b**128 + n (int32)
  Pool: batched indirect DMA gather (8 rows/partition per call) of
        256B fp16 [key|value] rows + linear write to HBM
Host re-splits the interleaved fp16 kv output into fp32 keys/values.
(GPSIMD/Pool compute is rejected by this neuronxcc build, so Pool only
issues DMAs; fp16 kv halves gather/write bytes at ~3e-4 value error.)
"""

import sys

import numpy as np

for _p in ("/opt/trn_rl_repo", "/root/.axon_site/_ro/trn_rl_repo"):
    if _p not in sys.path:
        sys.path.insert(0, _p)

NUM_QUERIES = 1024
NUM_DATASETS = 8
DB_SIZE = 32768
KEY_FEATURES = 64
VALUE_FEATURES = 64
NUM_NEIGHBORS = 128  # == num_buckets == n axis size

# tunables
SW = 2048            # super-tile width (columns per PSUM tile)
GW = 8               # gathers grouped per okv write

_NC_CACHE = {}


def build_nc(Q=NUM_QUERIES, DB=DB_SIZE):
    import concourse.bass as bass
    import concourse.mybir as mybir
    import concourse.tile as tile
    from concourse import bacc

    F = KEY_FEATURES
    NB = NUM_NEIGHBORS          # buckets per query chunk
    BS = DB // NB               # bucket size (argmax range) = 256
    KVW = 128                   # fp16 row: 64 key + 64 value
    NPST = SW // BS             # buckets per super-tile = 8
    NST = DB // SW              # super-tiles per q-chunk = 16
    QC = Q // 128               # q-chunks = 8
    NMM = SW // 512             # matmuls per super-tile = 4
    assert NB % NPST == 0 and NB % GW == 0

    nc = bacc.Bacc()
    qT = nc.declare_dram_parameter("qT", [F, Q], mybir.dt.float32, isOutput=False)
    kTp = nc.declare_dram_parameter("kTp", [F, DB], mybir.dt.float32, isOutput=False)
    kv = nc.declare_dram_parameter("kv", [DB, KVW], mybir.dt.float16, isOutput=False)
    biota = nc.declare_dram_parameter("biota", [128, SW], mybir.dt.bfloat16, isOutput=False)
    niota = nc.declare_dram_parameter("niota", [128, NB], mybir.dt.float32, isOutput=False)
    okv = nc.declare_dram_parameter("okv", [Q, NB, KVW], mybir.dt.float16, isOutput=True)

    X = mybir.AxisListType.X
    OP = mybir.AluOpType
    AF = mybir.ActivationFunctionType

    with tile.TileContext(nc) as tc:
        with (
            tc.tile_pool(name="const", bufs=1) as constp,
            tc.tile_pool(name="eqs", bufs=3) as eqp,
            tc.tile_pool(name="tr", bufs=2) as trp,
            tc.tile_pool(name="acc", bufs=3) as accp,
            tc.tile_pool(name="gkv", bufs=4) as gkvp,
            tc.tile_pool(name="ps", bufs=2, space="PSUM") as psp,
        ):
            qt = constp.tile([F, Q], mybir.dt.float32, tag="qt")
            nc.sync.dma_start(out=qt[:], in_=qT[:])
            kt = constp.tile([F, DB], mybir.dt.float32, tag="kt")
            for c in range(8):
                w = DB // 8
                nc.sync.dma_start(out=kt[:, c * w:(c + 1) * w],
                                  in_=kTp[:, c * w:(c + 1) * w])
            bio = constp.tile([128, SW], mybir.dt.bfloat16, tag="bio")
            nc.sync.dma_start(out=bio[:], in_=biota[:])
            nio = constp.tile([128, NB], mybir.dt.float32, tag="nio")
            nc.sync.dma_start(out=nio[:], in_=niota[:])

            for qc in range(QC):
                Mn = accp.tile([128, NB], mybir.dt.float32, tag="Mn")
                bsel = accp.tile([128, NB], mybir.dt.float32, tag="bsel")
                for st in range(NST):
                    ps = psp.tile([128, SW], mybir.dt.float32, tag="ps")
                    for j in range(NMM):
                        nc.tensor.matmul(
                            ps[:, j * 512:(j + 1) * 512],
                            qt[:, qc * 128:(qc + 1) * 128],
                            kt[:, st * SW + j * 512: st * SW + (j + 1) * 512],
                            start=True,
                            stop=True,
                        )
                    n0 = st * NPST
                    # J1: per-bucket max over the 256 candidates, negated
                    nc.vector.tensor_reduce(
                        Mn[:, n0:n0 + NPST],
                        ps[:].rearrange("p (n b) -> p n b", b=BS),
                        axis=X, op=OP.max, negate=True,
                    )
                    # J2: one-cold compare on ACT: Sign(s - M) in {-1, 0}
                    eq = eqp.tile([128, SW], mybir.dt.bfloat16, tag="eq")
                    for nl in range(NPST):
                        nc.scalar.activation(
                            out=eq[:, nl * BS:(nl + 1) * BS],
                            in_=ps[:, nl * BS:(nl + 1) * BS],
                            func=AF.Sign,
                            bias=Mn[:, n0 + nl:n0 + nl + 1],
                        )
                    # J3: fused extract: (eq'+1)*biota, accum-sum -> b*/256
                    trash = trp.tile([128, BS], mybir.dt.bfloat16, tag="trash")
                    for nl in range(NPST):
                        nc.vector.scalar_tensor_tensor(
                            out=trash[:],
                            in0=eq[:, nl * BS:(nl + 1) * BS],
                            scalar=1.0,
                            in1=bio[:, nl * BS:(nl + 1) * BS],
                            op0=OP.add, op1=OP.mult,
                            accum_out=bsel[:, n0 + nl:n0 + nl + 1],
                        )
                # decode: clamp ties, kv row = b*128 + n
                bc = accp.tile([128, NB], mybir.dt.float32, tag="bc")
                nc.vector.tensor_scalar(
                    out=bc[:], in0=bsel[:], scalar1=255.0 / 256.0, scalar2=None,
                    op0=OP.min,
                )
                offf = accp.tile([128, NB], mybir.dt.float32, tag="offf")
                nc.vector.scalar_tensor_tensor(
                    out=offf[:], in0=bc[:], scalar=32768.0, in1=nio[:],
                    op0=OP.mult, op1=OP.add,
                )
                offi = accp.tile([128, NB], mybir.dt.int32, tag="offi")
                nc.vector.tensor_copy(out=offi[:], in_=offf[:])
                # canonical gathers (1 row/partition per call; this NRT
                # ignores extra offsets), grouped GW per okv write
                for g in range(NB // GW):
                    gk = gkvp.tile([128, GW * KVW], mybir.dt.float16, tag="gk")
                    for j in range(GW):
                        n = g * GW + j
                        nc.gpsimd.indirect_dma_start(
                            out=gk[:, j * KVW:(j + 1) * KVW],
                            out_offset=None,
                            in_=kv[:],
                            in_offset=bass.IndirectOffsetOnAxis(
                                ap=offi[:, n:n + 1], axis=0
                            ),
                        )
                    nc.sync.dma_start(
                        out=okv[qc * 128:(qc + 1) * 128, g * GW:(g + 1) * GW, :],
                        in_=gk[:].rearrange("p (g f) -> p g f", f=KVW),
                    )
    if not nc.is_finalized():
        nc.finalize()
    return nc


def _get_nc(Q, DB):
    key = (Q, DB)
    if key not in _NC_CACHE:
        _NC_CACHE[key] = build_nc(Q, DB)
    return _NC_CACHE[key]


def make_core_inputs(query, key_db, value_db, d, Q, DB):
    """Host-side prep of one core's input arrays (dataset d)."""
    import ml_dtypes

    F = KEY_FEATURES
    NB = NUM_NEIGHBORS
    BS = DB // NB
    qTn = np.ascontiguousarray(query[:, d, :].T.astype(np.float32))          # (F, Q)
    # kTp[f, n*BS + b] = key_db[d, b*NB + n, f]  (bucket-inner column order)
    kperm = key_db[d].reshape(BS, NB, F).transpose(2, 1, 0)                   # (F, NB, BS)
    kTpn = np.ascontiguousarray(kperm.reshape(F, NB * BS).astype(np.float32))
    kvn = np.ascontiguousarray(
        np.concatenate([key_db[d], value_db[d]], axis=1).astype(np.float16)  # (DB, 128)
    )
    bio01 = ((np.arange(SW) % BS).astype(np.float32) / 256.0).astype(ml_dtypes.bfloat16)
    bio = np.broadcast_to(bio01, (128, SW)).copy()
    nio = np.broadcast_to(np.arange(NB, dtype=np.float32), (128, NB)).copy()
    return {"qT": qTn, "kTp": kTpn, "kv": kvn, "biota": bio, "niota": nio}


def kernel(query, key_db, value_db, num_neighbors):
    from concourse.bass_utils import run_bass_kernel_spmd

    query = np.asarray(query, dtype=np.float32)
    key_db = np.asarray(key_db, dtype=np.float32)
    value_db = np.asarray(value_db, dtype=np.float32)
    assert int(num_neighbors) == NUM_NEIGHBORS
    Q, D, F = query.shape
    _, DB, _ = key_db.shape
    assert (Q, D, F, DB) == (NUM_QUERIES, NUM_DATASETS, KEY_FEATURES, DB_SIZE)

    nc = _get_nc(Q, DB)
    in_maps = [make_core_inputs(query, key_db, value_db, d, Q, DB) for d in range(D)]
    res = run_bass_kernel_spmd(nc, in_maps, core_ids=list(range(D)))

    sel_k = np.empty((Q, D, NUM_NEIGHBORS, KEY_FEATURES), dtype=np.float32)
    sel_v = np.empty((Q, D, NUM_NEIGHBORS, VALUE_FEATURES), dtype=np.float32)
    for d in range(D):
        okv = np.asarray(res.results[d]["okv"], dtype=np.float32)  # (Q, NB, 128)
        sel_k[:, d] = okv[:, :, :KEY_FEATURES]
        sel_v[:, d] = okv[:, :, KEY_FEATURES:]
    return sel_k, sel_v


# revision 7
# speedup vs baseline: 3.3756x; 1.6222x over previous
"""Trainium2 Bass kernel for nn_MemoryOnGpu (retrieval_knn).

Reference semantics: for each (query q, dataset d, bucket n):
  pick b* = argmax_b  <query[q,d,:], key_db[d, b*128+n, :]>   (b in [0, 256))
  selected_keys[q,d,n,:]   = key_db[d, b**128+n, :]
  selected_values[q,d,n,:] = value_db[d, b**128+n, :]

Sharding: one dataset per NeuronCore (8 datasets, 8 cores).

Per-core pipeline, per 128-query chunk, per 2048-column super-tile
(8 buckets, columns in bucket-inner order so each bucket's 256
candidates are contiguous):
  PE  : bf16 split-K scores = [qh;ql]@[kh;kh] (K=128) + qh@kl (K=64),
        accumulated in fp32 PSUM -- fp32-grade argmax accuracy at bf16 rate
  DVE : segmented reduce-max, negated    -> Mneg[q, n]
  ACT : per-bucket Sign(scores + Mneg)   -> eq' in {-1, 0}   (0 at argmax)
  DVE : per-bucket fused STT (eq'+1)*biota with accum-sum -> bsel = b*/256
  DVE : clamp + decode to kv row index b**128 + n (int32), per super-tile
  Pool: 8 canonical indirect gathers per super-tile (1 row/partition each;
        this NRT ignores extra offsets) of 256B fp16 [key|value] rows,
        round-robined over 4 SWDGE queues for transfer parallelism
  SP/ACT/DVE: okv writes round-robined so no single DMA ring throttles
Host re-splits the interleaved fp16 kv output into fp32 keys/values.
(GPSIMD/Pool compute and dma_gather ucode are unavailable on this image,
so Pool only issues canonical one-offset-per-partition indirect DMAs.)
"""

import sys

import numpy as np

for _p in ("/opt/trn_rl_repo", "/root/.axon_site/_ro/trn_rl_repo"):
    if _p not in sys.path:
        sys.path.insert(0, _p)

NUM_QUERIES = 1024
NUM_DATASETS = 8
DB_SIZE = 32768
KEY_FEATURES = 64
VALUE_FEATURES = 64
NUM_NEIGHBORS = 128  # == num_buckets == n axis size

SW = 2048            # super-tile width (columns per PSUM tile)
NSWQ = 4             # SWDGE queues for gather round-robin

_NC_CACHE = {}


def build_nc(Q=NUM_QUERIES, DB=DB_SIZE):
    import concourse.bass as bass
    import concourse.mybir as mybir
    import concourse.tile as tile
    from concourse import bacc

    F = KEY_FEATURES
    NB = NUM_NEIGHBORS          # buckets per query chunk
    BS = DB // NB               # bucket size (argmax range) = 256
    KVW = 128                   # fp16 row: 64 key + 64 value
    NPST = SW // BS             # buckets per super-tile = 8
    NST = DB // SW              # super-tiles per q-chunk = 16
    QC = Q // 128               # q-chunks = 8
    NMM = SW // 512             # 512-col groups per super-tile = 4
    assert NB % NPST == 0

    nc = bacc.Bacc(num_swdge_queues=NSWQ)
    # split-K stationary: [qh; ql] stacked on 128 partitions
    qhl = nc.declare_dram_parameter("qhl", [2 * F, Q], mybir.dt.bfloat16, isOutput=False)
    # moving pass 1: [kh; kh] stacked on 128 partitions
    khh = nc.declare_dram_parameter("khh", [2 * F, DB], mybir.dt.bfloat16, isOutput=False)
    # moving pass 2: kl on 64 partitions
    klo = nc.declare_dram_parameter("klo", [F, DB], mybir.dt.bfloat16, isOutput=False)
    kv = nc.declare_dram_parameter("kv", [DB, KVW], mybir.dt.float16, isOutput=False)
    biota = nc.declare_dram_parameter("biota", [128, SW], mybir.dt.bfloat16, isOutput=False)
    niota = nc.declare_dram_parameter("niota", [128, NB], mybir.dt.float32, isOutput=False)
    okv = nc.declare_dram_parameter("okv", [Q, NB, KVW], mybir.dt.float16, isOutput=True)

    X = mybir.AxisListType.X
    OP = mybir.AluOpType
    AF = mybir.ActivationFunctionType

    with tile.TileContext(nc) as tc:
        with (
            tc.tile_pool(name="const", bufs=1) as constp,
            tc.tile_pool(name="eqs", bufs=3) as eqp,
            tc.tile_pool(name="tr", bufs=2) as trp,
            tc.tile_pool(name="sel", bufs=4) as selp,
            tc.tile_pool(name="acc", bufs=2) as accp,
            tc.tile_pool(name="gkv", bufs=5) as gkvp,
            tc.tile_pool(name="ps", bufs=2, space="PSUM") as psp,
        ):
            qt = constp.tile([2 * F, Q], mybir.dt.bfloat16, tag="qt")
            nc.sync.dma_start(out=qt[:], in_=qhl[:])
            kh = constp.tile([2 * F, DB], mybir.dt.bfloat16, tag="kh")
            for c in range(8):
                w = DB // 8
                nc.sync.dma_start(out=kh[:, c * w:(c + 1) * w],
                                  in_=khh[:, c * w:(c + 1) * w])
            kl = constp.tile([F, DB], mybir.dt.bfloat16, tag="kl")
            for c in range(4):
                w = DB // 4
                nc.sync.dma_start(out=kl[:, c * w:(c + 1) * w],
                                  in_=klo[:, c * w:(c + 1) * w])
            bio = constp.tile([128, SW], mybir.dt.bfloat16, tag="bio")
            nc.sync.dma_start(out=bio[:], in_=biota[:])
            nio = constp.tile([128, NB], mybir.dt.float32, tag="nio")
            nc.sync.dma_start(out=nio[:], in_=niota[:])

            wr_engines = [nc.sync, nc.scalar]
            gq = 0  # SWDGE queue rotation
            for qc in range(QC):
                Mn = accp.tile([128, NB], mybir.dt.float32, tag="Mn")
                for st in range(NST):
                    ps = psp.tile([128, SW], mybir.dt.float32, tag="ps")
                    for j in range(NMM):
                        c0 = st * SW + j * 512
                        nc.tensor.matmul(
                            ps[:, j * 512:(j + 1) * 512],
                            qt[:, qc * 128:(qc + 1) * 128],
                            kh[:, c0:c0 + 512],
                            start=True, stop=False,
                        )
                        nc.tensor.matmul(
                            ps[:, j * 512:(j + 1) * 512],
                            qt[0:F, qc * 128:(qc + 1) * 128],
                            kl[:, c0:c0 + 512],
                            start=False, stop=True,
                        )
                    n0 = st * NPST
                    # J1: per-bucket max over the 256 candidates, negated
                    nc.vector.tensor_reduce(
                        Mn[:, n0:n0 + NPST],
                        ps[:].rearrange("p (n b) -> p n b", b=BS),
                        axis=X, op=OP.max, negate=True,
                    )
                    # J2: one-cold compare on ACT: Sign(s - M) in {-1, 0}
                    eq = eqp.tile([128, SW], mybir.dt.bfloat16, tag="eq")
                    for nl in range(NPST):
                        nc.scalar.activation(
                            out=eq[:, nl * BS:(nl + 1) * BS],
                            in_=ps[:, nl * BS:(nl + 1) * BS],
                            func=AF.Sign,
                            bias=Mn[:, n0 + nl:n0 + nl + 1],
                        )
                    # J3: fused extract: (eq'+1)*biota, accum-sum -> b*/256
                    bsel = selp.tile([128, NPST], mybir.dt.float32, tag="bsel")
                    trash = trp.tile([128, BS], mybir.dt.bfloat16, tag="trash")
                    for nl in range(NPST):
                        nc.vector.scalar_tensor_tensor(
                            out=trash[:],
                            in0=eq[:, nl * BS:(nl + 1) * BS],
                            scalar=1.0,
                            in1=bio[:, nl * BS:(nl + 1) * BS],
                            op0=OP.add, op1=OP.mult,
                            accum_out=bsel[:, nl:nl + 1],
                        )
                    # decode this super-tile: clamp ties, kv row = b*128 + n
                    bc = selp.tile([128, NPST], mybir.dt.float32, tag="bc")
                    nc.vector.tensor_scalar(
                        out=bc[:], in0=bsel[:], scalar1=255.0 / 256.0,
                        scalar2=None, op0=OP.min,
                    )
                    offf = selp.tile([128, NPST], mybir.dt.float32, tag="offf")
                    nc.vector.scalar_tensor_tensor(
                        out=offf[:], in0=bc[:], scalar=32768.0,
                        in1=nio[:, n0:n0 + NPST],
                        op0=OP.mult, op1=OP.add,
                    )
                    offi = selp.tile([128, NPST], mybir.dt.int32, tag="offi")
                    nc.vector.tensor_copy(out=offi[:], in_=offf[:])
                    # gather this super-tile's 8 buckets + 1 okv write
                    gk = gkvp.tile([128, NPST * KVW], mybir.dt.float16, tag="gk")
                    for nl in range(NPST):
                        gi = nc.gpsimd.indirect_dma_start(
                            out=gk[:, nl * KVW:(nl + 1) * KVW],
                            out_offset=None,
                            in_=kv[:],
                            in_offset=bass.IndirectOffsetOnAxis(
                                ap=offi[:, nl:nl + 1], axis=0
                            ),
                        )
                        gi.ins.queue = f"qPoolDynamic{gq or ''}"
                        gq = (gq + 1) % NSWQ
                    wr_engines[st % 2].dma_start(
                        out=okv[qc * 128:(qc + 1) * 128, n0:n0 + NPST, :],
                        in_=gk[:].rearrange("p (g f) -> p g f", f=KVW),
                    )
    if not nc.is_finalized():
        nc.finalize()
    return nc


def _get_nc(Q, DB):
    key = (Q, DB)
    if key not in _NC_CACHE:
        _NC_CACHE[key] = build_nc(Q, DB)
    return _NC_CACHE[key]


def make_core_inputs(query, key_db, value_db, d, Q, DB):
    """Host-side prep of one core's input arrays (dataset d)."""
    import ml_dtypes

    F = KEY_FEATURES
    NB = NUM_NEIGHBORS
    BS = DB // NB
    bf16 = ml_dtypes.bfloat16
    qTn = query[:, d, :].T.astype(np.float32)                                 # (F, Q)
    qh = qTn.astype(bf16)
    ql = (qTn - qh.astype(np.float32)).astype(bf16)
    qhl = np.ascontiguousarray(np.concatenate([qh, ql], axis=0))              # (2F, Q)
    # kTp[f, n*BS + b] = key_db[d, b*NB + n, f]  (bucket-inner column order)
    kperm = key_db[d].reshape(BS, NB, F).transpose(2, 1, 0)                   # (F, NB, BS)
    kTpn = kperm.reshape(F, NB * BS).astype(np.float32)
    kh = kTpn.astype(bf16)
    kl = (kTpn - kh.astype(np.float32)).astype(bf16)
    khh = np.ascontiguousarray(np.concatenate([kh, kh], axis=0))              # (2F, DB)
    klo = np.ascontiguousarray(kl)                                            # (F, DB)
    kvn = np.ascontiguousarray(
        np.concatenate([key_db[d], value_db[d]], axis=1).astype(np.float16)  # (DB, 128)
    )
    bio01 = ((np.arange(SW) % BS).astype(np.float32) / 256.0).astype(bf16)
    bio = np.broadcast_to(bio01, (128, SW)).copy()
    nio = np.broadcast_to(np.arange(NB, dtype=np.float32), (128, NB)).copy()
    return {"qhl": qhl, "khh": khh, "klo": klo, "kv": kvn, "biota": bio,
            "niota": nio}


def kernel(query, key_db, value_db, num_neighbors):
    from concourse.bass_utils import run_bass_kernel_spmd

    query = np.asarray(query, dtype=np.float32)
    key_db = np.asarray(key_db, dtype=np.float32)
    value_db = np.asarray(value_db, dtype=np.float32)
    assert int(num_neighbors) == NUM_NEIGHBORS
    Q, D, F = query.shape
    _, DB, _ = key_db.shape
    assert (Q, D, F, DB) == (NUM_QUERIES, NUM_DATASETS, KEY_FEATURES, DB_SIZE)

    nc = _get_nc(Q, DB)
    in_maps = [make_core_inputs(query, key_db, value_db, d, Q, DB) for d in range(D)]
    res = run_bass_kernel_spmd(nc, in_maps, core_ids=list(range(D)))

    sel_k = np.empty((Q, D, NUM_NEIGHBORS, KEY_FEATURES), dtype=np.float32)
    sel_v = np.empty((Q, D, NUM_NEIGHBORS, VALUE_FEATURES), dtype=np.float32)
    for d in range(D):
        okv = np.asarray(res.results[d]["okv"], dtype=np.float32)  # (Q, NB, 128)
        sel_k[:, d] = okv[:, :, :KEY_FEATURES]
        sel_v[:, d] = okv[:, :, KEY_FEATURES:]
    return sel_k, sel_v


# revision 8
# speedup vs baseline: 3.7850x; 1.1213x over previous
"""Trainium2 Bass kernel for nn_MemoryOnGpu (retrieval_knn) — hybrid v3.

Per (query q, dataset d, bucket n): pick b* = argmax_b <q, key_db[b*128+n]>,
output that key and value row. One dataset per core.

The SWDGE indirect-DMA path costs ~1.4us per 128-row gather call on the Pool
engine (descgen is Pool-serial), so a full-gather kernel floors at ~1.45ms.
v3 splits each super-tile's buckets between two retrieval paths:
  - GSEL buckets/st: canonical indirect gathers (Pool), as in v2
  - the rest: on-chip PE selection — transpose the one-cold mask, +1 to
    one-hot while copying PSUM->SBUF, then sel = onehot^T @ kv2 (K=128 x2)
Scores: bf16 split-K ([ql;qh]@[kh;kl] one K=128 pass = both cross terms,
+ qh@kh K=64 second pass, PSUM-accumulated) = fp32-grade argmax accuracy.
Argmax: DVE segmented reduce-max (negated) -> ACT Sign(s+Mneg) in {-1,0}
-> (gather path) DVE STT (eq'+1)*biota accum-sum -> row index
-> (select path) PE transpose + ACT(+1 copy) + PE select matmuls.
"""

import sys

import numpy as np

for _p in ("/opt/trn_rl_repo", "/root/.axon_site/_ro/trn_rl_repo"):
    if _p not in sys.path:
        sys.path.insert(0, _p)

NUM_QUERIES = 1024
NUM_DATASETS = 8
DB_SIZE = 32768
KEY_FEATURES = 64
VALUE_FEATURES = 64
NUM_NEIGHBORS = 128

SW = 1024            # super-tile width (2 PSUM banks)
GSEL = 2             # buckets per super-tile on the gather path (of SW/256)
NSWQ = 4             # SWDGE queues for gather round-robin

_NC_CACHE = {}


def build_nc(Q=NUM_QUERIES, DB=DB_SIZE):
    import concourse.bass as bass
    import concourse.mybir as mybir
    import concourse.tile as tile
    from concourse import bacc

    F = KEY_FEATURES
    NB = NUM_NEIGHBORS
    BS = DB // NB               # 256 candidates per bucket
    KVW = 128                   # fp16 row: 64 key + 64 value
    NPST = SW // BS             # buckets per super-tile = 4
    NST = DB // SW              # super-tiles per q-chunk = 32
    QC = Q // 128               # q-chunks = 8
    NMM = SW // 512             # 512-col groups per super-tile = 2
    NSEL = NPST - GSEL          # PE-selected buckets per super-tile
    assert 0 < GSEL < NPST

    nc = bacc.Bacc(num_swdge_queues=NSWQ)
    # scores split-K operands: [ql; qh] stationary, [kh; kl] moving
    qlh = nc.declare_dram_parameter("qlh", [2 * F, Q], mybir.dt.bfloat16, isOutput=False)
    khkl = nc.declare_dram_parameter("khkl", [2 * F, DB], mybir.dt.bfloat16, isOutput=False)
    kv = nc.declare_dram_parameter("kv", [DB, KVW], mybir.dt.float16, isOutput=False)
    # kv2[p, n*2+bh, f] = kv[(bh*128+p)*128 + n, f]  (select-path table)
    kv2 = nc.declare_dram_parameter("kv2", [128, 2 * NB * KVW], mybir.dt.float16, isOutput=False)
    biota = nc.declare_dram_parameter("biota", [128, SW], mybir.dt.float16, isOutput=False)
    niota = nc.declare_dram_parameter("niota", [128, NB], mybir.dt.float32, isOutput=False)
    ident = nc.declare_dram_parameter("ident", [128, 128], mybir.dt.float16, isOutput=False)
    okv = nc.declare_dram_parameter("okv", [Q, NB, KVW], mybir.dt.float16, isOutput=True)

    X = mybir.AxisListType.X
    OP = mybir.AluOpType
    AF = mybir.ActivationFunctionType

    with tile.TileContext(nc) as tc:
        with (
            tc.tile_pool(name="const", bufs=1) as constp,
            tc.tile_pool(name="eqs", bufs=4) as eqp,
            tc.tile_pool(name="tr", bufs=2) as trp,
            tc.tile_pool(name="sel", bufs=4) as selp,
            tc.tile_pool(name="onh", bufs=4) as onhp,
            tc.tile_pool(name="selo", bufs=4) as selop,
            tc.tile_pool(name="acc", bufs=2) as accp,
            tc.tile_pool(name="gkv", bufs=6) as gkvp,
            tc.tile_pool(name="ps", bufs=3, space="PSUM") as psp,
            tc.tile_pool(name="pt", bufs=1, space="PSUM") as ptp,
            tc.tile_pool(name="po", bufs=1, space="PSUM") as pop,
        ):
            qt = constp.tile([2 * F, Q], mybir.dt.bfloat16, tag="qt")
            nc.sync.dma_start(out=qt[:], in_=qlh[:])
            qh2 = constp.tile([F, Q], mybir.dt.bfloat16, tag="qh2")
            nc.sync.dma_start(out=qh2[:], in_=qlh[F:2 * F, :])
            kt = constp.tile([2 * F, DB], mybir.dt.bfloat16, tag="kt")
            for c in range(8):
                w = DB // 8
                nc.sync.dma_start(out=kt[:, c * w:(c + 1) * w],
                                  in_=khkl[:, c * w:(c + 1) * w])
            k2 = constp.tile([128, 2 * NB * KVW], mybir.dt.float16, tag="k2")
            for c in range(8):
                w = 2 * NB * KVW // 8
                nc.sync.dma_start(out=k2[:, c * w:(c + 1) * w],
                                  in_=kv2[:, c * w:(c + 1) * w])
            bio = constp.tile([128, SW], mybir.dt.float16, tag="bio")
            nc.sync.dma_start(out=bio[:], in_=biota[:])
            nio = constp.tile([128, NB], mybir.dt.float32, tag="nio")
            nc.sync.dma_start(out=nio[:], in_=niota[:])
            idt = constp.tile([128, 128], mybir.dt.float16, tag="idt")
            nc.sync.dma_start(out=idt[:], in_=ident[:])

            gq = 0
            for qc in range(QC):
                Mn = accp.tile([128, NB], mybir.dt.float32, tag="Mn")
                qs = qt[:, qc * 128:(qc + 1) * 128]
                qh_only = qh2[:, qc * 128:(qc + 1) * 128]
                for st in range(NST):
                    gsel = 3 if st % 4 == 0 else GSEL
                    nsel = NPST - gsel
                    ps = psp.tile([128, SW], mybir.dt.float32, tag="ps")
                    for j in range(NMM):
                        c0 = st * SW + j * 512
                        # pass 1 (K=128): ql@kh + qh@kl
                        nc.tensor.matmul(
                            ps[:, j * 512:(j + 1) * 512], qs,
                            kt[:, c0:c0 + 512], start=True, stop=False,
                        )
                        # pass 2 (K=64): qh@kh
                        nc.tensor.matmul(
                            ps[:, j * 512:(j + 1) * 512], qh_only,
                            kt[0:F, c0:c0 + 512], start=False, stop=True,
                        )
                    n0 = st * NPST
                    nc.vector.tensor_reduce(
                        Mn[:, n0:n0 + NPST],
                        ps[:].rearrange("p (n b) -> p n b", b=BS),
                        axis=X, op=OP.max, negate=True,
                    )
                    eq = eqp.tile([128, SW], mybir.dt.float16, tag="eq")
                    for nl in list(range(gsel, NPST)) + list(range(gsel)):
                        nc.scalar.activation(
                            out=eq[:, nl * BS:(nl + 1) * BS],
                            in_=ps[:, nl * BS:(nl + 1) * BS],
                            func=AF.Sign,
                            bias=Mn[:, n0 + nl:n0 + nl + 1],
                        )
                    # ---- gather path: buckets n0 .. n0+GSEL-1 ----
                    bsel = selp.tile([128, 3], mybir.dt.float32, tag="bsel")
                    trash = trp.tile([128, BS], mybir.dt.float16, tag="trash")
                    for nl in range(gsel):
                        nc.vector.scalar_tensor_tensor(
                            out=trash[:],
                            in0=eq[:, nl * BS:(nl + 1) * BS],
                            scalar=1.0,
                            in1=bio[:, nl * BS:(nl + 1) * BS],
                            op0=OP.add, op1=OP.mult,
                            accum_out=bsel[:, nl:nl + 1],
                        )
                    offf = selp.tile([128, 3], mybir.dt.float32, tag="offf")
                    nc.vector.scalar_tensor_tensor(
                        out=offf[:, 0:gsel], in0=bsel[:, 0:gsel], scalar=32640.0,
                        in1=nio[:, n0:n0 + gsel],
                        op0=OP.min, op1=OP.add,
                    )
                    offi = selp.tile([128, 3], mybir.dt.int32, tag="offi")
                    nc.vector.tensor_copy(out=offi[:, 0:gsel], in_=offf[:, 0:gsel])
                    gk = gkvp.tile([128, 3 * KVW], mybir.dt.float16, tag="gk")
                    for nl in range(gsel):
                        gi = nc.gpsimd.indirect_dma_start(
                            out=gk[:, nl * KVW:(nl + 1) * KVW],
                            out_offset=None,
                            in_=kv[:],
                            in_offset=bass.IndirectOffsetOnAxis(
                                ap=offi[:, nl:nl + 1], axis=0
                            ),
                        )
                        gi.ins.queue = f"qPoolDynamic{gq or ''}"
                        gq = (gq + 1) % NSWQ
                    nc.sync.dma_start(
                        out=okv[qc * 128:(qc + 1) * 128, n0:n0 + gsel, :],
                        in_=gk[:, 0:gsel * KVW].rearrange("p (g f) -> p g f", f=KVW),
                    )
                    # ---- select path: buckets n0+GSEL .. n0+NPST-1 ----
                    eqT = ptp.tile([128, NSEL * BS], mybir.dt.float16, tag="eqT")
                    for si in range(nsel):
                        nl = gsel + si
                        for bh in range(2):
                            nc.tensor.transpose(
                                eqT[:, (si * 2 + bh) * 128:(si * 2 + bh + 1) * 128],
                                eq[:, nl * BS + bh * 128:nl * BS + (bh + 1) * 128],
                                idt[:],
                            )
                    onh = onhp.tile([128, NSEL * BS], mybir.dt.float16, tag="onh")
                    w = nsel * BS
                    if st % 2 == 0:
                        nc.scalar.activation(
                            out=onh[:, 0:w], in_=eqT[:, 0:w], func=AF.Identity, bias=1.0,
                        )
                    else:
                        nc.vector.tensor_scalar(
                            out=onh[:, 0:w], in0=eqT[:, 0:w], scalar1=1.0, scalar2=None,
                            op0=OP.add,
                        )
                    selps = pop.tile([128, NSEL * KVW], mybir.dt.float32, tag="selps")
                    for si in range(nsel):
                        n = n0 + gsel + si
                        for bh in range(2):
                            nc.tensor.matmul(
                                selps[:, si * KVW:(si + 1) * KVW],
                                onh[:, (si * 2 + bh) * 128:(si * 2 + bh + 1) * 128],
                                k2[:, (n * 2 + bh) * KVW:(n * 2 + bh + 1) * KVW],
                                start=(bh == 0), stop=(bh == 1),
                            )
                    selo = selop.tile([128, NSEL * KVW], mybir.dt.float16, tag="selo")
                    w2 = nsel * KVW
                    if st % 2 == 0:
                        nc.scalar.activation(out=selo[:, 0:w2], in_=selps[:, 0:w2], func=AF.Copy)
                    else:
                        nc.vector.tensor_copy(out=selo[:, 0:w2], in_=selps[:, 0:w2])
                    nc.sync.dma_start(
                        out=okv[qc * 128:(qc + 1) * 128, n0 + gsel:n0 + NPST, :],
                        in_=selo[:, 0:w2].rearrange("p (g f) -> p g f", f=KVW),
                    )
    if not nc.is_finalized():
        nc.finalize()
    return nc


def _get_nc(Q, DB):
    key = (Q, DB)
    if key not in _NC_CACHE:
        _NC_CACHE[key] = build_nc(Q, DB)
    return _NC_CACHE[key]


def make_core_inputs(query, key_db, value_db, d, Q, DB):
    """Host-side prep of one core's input arrays (dataset d)."""
    import ml_dtypes

    F = KEY_FEATURES
    NB = NUM_NEIGHBORS
    BS = DB // NB
    KVW = 128
    bf16 = ml_dtypes.bfloat16
    qTn = query[:, d, :].T.astype(np.float32)                                 # (F, Q)
    qh = qTn.astype(bf16)
    ql = (qTn - qh.astype(np.float32)).astype(bf16)
    qlh = np.ascontiguousarray(np.concatenate([ql, qh], axis=0))              # (2F, Q)
    kperm = key_db[d].reshape(BS, NB, F).transpose(2, 1, 0)                   # (F, NB, BS)
    kTpn = kperm.reshape(F, NB * BS).astype(np.float32)
    kh = kTpn.astype(bf16)
    kl = (kTpn - kh.astype(np.float32)).astype(bf16)
    khkl = np.ascontiguousarray(np.concatenate([kh, kl], axis=0))             # (2F, DB)
    kvn = np.ascontiguousarray(
        np.concatenate([key_db[d], value_db[d]], axis=1).astype(np.float16)  # (DB, 128)
    )
    # kv2[p, n*2+bh, f] = kvn[(bh*128+p)*128 + n, f]
    kv2 = kvn.reshape(2, 128, NB, KVW)            # (bh, p, n, f)
    kv2 = np.ascontiguousarray(
        kv2.transpose(1, 2, 0, 3).reshape(128, 2 * NB * KVW)
    )
    bio01 = ((np.arange(SW) % BS).astype(np.float32) * 128.0).astype(np.float16)
    bio = np.broadcast_to(bio01, (128, SW)).copy()
    nio = np.broadcast_to(np.arange(NB, dtype=np.float32), (128, NB)).copy()
    identm = np.eye(128, dtype=np.float16)
    return {"qlh": qlh, "khkl": khkl, "kv": kvn, "kv2": kv2, "biota": bio,
            "niota": nio, "ident": identm}


def kernel(query, key_db, value_db, num_neighbors):
    from concourse.bass_utils import run_bass_kernel_spmd

    query = np.asarray(query, dtype=np.float32)
    key_db = np.asarray(key_db, dtype=np.float32)
    value_db = np.asarray(value_db, dtype=np.float32)
    assert int(num_neighbors) == NUM_NEIGHBORS
    Q, D, F = query.shape
    _, DB, _ = key_db.shape
    assert (Q, D, F, DB) == (NUM_QUERIES, NUM_DATASETS, KEY_FEATURES, DB_SIZE)

    nc = _get_nc(Q, DB)
    in_maps = [make_core_inputs(query, key_db, value_db, d, Q, DB) for d in range(D)]
    res = run_bass_kernel_spmd(nc, in_maps, core_ids=list(range(D)))

    sel_k = np.empty((Q, D, NUM_NEIGHBORS, KEY_FEATURES), dtype=np.float32)
    sel_v = np.empty((Q, D, NUM_NEIGHBORS, VALUE_FEATURES), dtype=np.float32)
    for d in range(D):
        okv = np.asarray(res.results[d]["okv"], dtype=np.float32)
        sel_k[:, d] = okv[:, :, :KEY_FEATURES]
        sel_v[:, d] = okv[:, :, KEY_FEATURES:]
    return sel_k, sel_v
